# revision 1
# baseline (speedup 1.0000x reference)
"""CenterLoss kernel for Trainium2, data-parallel over 8 NeuronCores.

loss = sum(clip(distmat * onehot(argmax(logits)), 1e-12, 1e12)) / N
     = (sum_i clip(||f_i - c_{label_i}||^2, 1e-12, 1e12) + N*(C-1)*1e-12) / N

Each core handles 640 samples: per 128-row tile it computes a chunked
argmax over the 6625 logits (one full DVE pass -> 53 chunk maxima,
top-8 max/max_index on those, then an indirect-DMA re-fetch of the
winning 125-wide chunk for the exact index), gathers each sample's
center row by label via indirect DMA, and reduces ||f-c||^2 per sample.
Host gathers the 8x640 per-sample distances and finishes the scalar.
"""

import numpy as np

import concourse.bacc as bacc
import concourse.bass as bass
import concourse.tile as tile
from concourse import mybir
from concourse.bass_utils import run_bass_kernel_spmd

P = 128          # SBUF partitions
C = 6625         # num classes
D = 96           # feat dim
K = 53           # chunks per row
S = 125          # chunk size (K * S == C)
T = 5            # 128-row tiles per core
ROWS = P * T     # 640 samples per core
N_CORES = 8
N = ROWS * N_CORES  # 5120 total samples
CLIP_MIN = 1e-12
CLIP_MAX = 1e12

f32 = mybir.dt.float32
u32 = mybir.dt.uint32
OP = mybir.AluOpType


def _build_nc():
    nc = bacc.Bacc(None)
    lg = nc.dram_tensor("logits", [ROWS, C], f32, kind="ExternalInput")
    ft = nc.dram_tensor("feats", [ROWS, D], f32, kind="ExternalInput")
    ct = nc.dram_tensor("centers", [C, D], f32, kind="ExternalInput")
    do = nc.dram_tensor("dout", [P, T], f32, kind="ExternalOutput")

    # Flat [ROWS*K, S] view of logits for gathering one chunk per sample.
    lg_flat = bass.AP(lg, 0, [[S, ROWS * K], [1, S]])

    with tile.TileContext(nc) as tc:
        with (
            tc.tile_pool(name="big", bufs=T) as big,
            tc.tile_pool(name="med", bufs=T) as med,
            tc.tile_pool(name="small", bufs=T) as small,
            tc.tile_pool(name="persist", bufs=1) as persist,
        ):
            dsum = persist.tile([P, T], f32)
            # riota_t[p, t] = t*P*K + p*K: per-tile row offset into lg_flat
            riota_t = persist.tile([P, T], u32)
            nc.gpsimd.iota(riota_t[:], [[P * K, T]], base=0, channel_multiplier=K)
            cS = persist.tile([P, 1], u32)
            nc.gpsimd.memset(cS[:], S)

            Fbig = persist.tile([P, T * D], f32)

            # Column-split each tile's load across the two HWDGE queues so
            # transfers overlap and each reduce starts as soon as its half
            # lands; finer split on first/last tiles shrinks the pipeline
            # fill (first reduce) and drain (tail reduce).
            engines = [nc.sync, nc.scalar]
            for t in range(T):
                L = big.tile([P, C], f32, tag="L")
                CM = small.tile([P, K], f32, tag="CM")
                if t == 0:
                    splits = [(0, 2), (2, 9), (9, 19), (19, 30), (30, 41), (41, K)]
                elif t == 1:
                    splits = [(0, 13), (13, 27), (27, 40), (40, K)]
                elif t < T - 1:
                    splits = [(0, 27), (27, K)]
                else:
                    splits = [(0, 14), (14, 27), (27, 40), (40, 50), (50, K)]
                for i, (k0, k1) in enumerate(splits):
                    eng = engines[(t + i) % 2]
                    eng.dma_start(
                        out=L[:, k0 * S:k1 * S],
                        in_=lg[t * P:(t + 1) * P, k0 * S:k1 * S],
                    )
                    nc.vector.reduce_max(
                        CM[:, k0:k1],
                        L[:, k0 * S:k1 * S].rearrange("p (k s) -> p k s", s=S),
                        axis=mybir.AxisListType.X,
                    )
                if t == 0:
                    # All feats in one DMA ([128, 5, 96] view of [640, 96]),
                    # issued behind the first tile's loads.
                    ft3 = bass.AP(ft, 0, [[D, P], [P * D, T], [1, D]])
                    nc.scalar.dma_start(
                        out=Fbig[:].rearrange("p (t d) -> p t d", d=D), in_=ft3
                    )
                # The per-tile argmax/gather/distance chain is latency- not
                # throughput-bound: raise its priority so it interleaves with
                # later tiles' reduces instead of queueing behind them.
                with tc.high_priority():
                    # Global max (top-8, col 0) and winning chunk index
                    GM8 = small.tile([P, 8], f32, tag="GM8")
                    nc.vector.max(out=GM8[:], in_=CM[:])
                    CI8 = small.tile([P, 8], u32, tag="CI8")
                    nc.vector.max_index(CI8[:], GM8[:], CM[:])

                    # Row index into lg_flat: (t*P + p)*K + chunk_idx.
                    # u32 operands are exact through the fp32 ALU (< 2^24).
                    RIu = small.tile([P, 1], u32, tag="RIu")
                    nc.gpsimd.tensor_add(
                        RIu[:], CI8[:, 0:1], riota_t[:, t:t + 1]
                    )

                    # Re-fetch each sample's winning chunk [P, S]
                    W = med.tile([P, S], f32, tag="W")
                    nc.gpsimd.indirect_dma_start(
                        out=W[:], out_offset=None, in_=lg_flat,
                        in_offset=bass.IndirectOffsetOnAxis(ap=RIu[:, :1], axis=0),
                    )
                    LI8 = small.tile([P, 8], u32, tag="LI8")
                    nc.vector.max_index(LI8[:], GM8[:], W[:])

                    # label = chunk_idx * S + local_idx, built on Pool so the
                    # whole CR-trigger chain stays on one engine. CIS is off
                    # the critical path (ready as soon as CI8 is).
                    CIS = small.tile([P, 1], u32, tag="CIS")
                    nc.gpsimd.tensor_mul(CIS[:], CI8[:, 0:1], cS[:])
                    LBu = small.tile([P, 1], u32, tag="LBu")
                    nc.gpsimd.tensor_add(LBu[:], CIS[:], LI8[:, 0:1])

                    # Gather each sample's center row
                    CR = med.tile([P, D], f32, tag="CR")
                    nc.gpsimd.indirect_dma_start(
                        out=CR[:], out_offset=None, in_=ct[:],
                        in_offset=bass.IndirectOffsetOnAxis(ap=LBu[:, :1], axis=0),
                    )

                # End of chain: d = sum((f - c)^2) into dsum[:, t]. Emitted
                # at default (low) priority — these wait on the CR gather and
                # must not head-of-line-block later tiles' chain ops on the
                # in-order engines.
                DF = med.tile([P, D], f32, tag="DF")
                nc.gpsimd.tensor_sub(DF[:], Fbig[:, t * D:(t + 1) * D], CR[:])
                SQ = med.tile([P, D], f32, tag="SQ")
                nc.vector.scalar_tensor_tensor(
                    out=SQ[:], in0=DF[:], scalar=0.0, in1=DF[:],
                    op0=OP.add, op1=OP.mult,
                    accum_out=dsum[:, t:t + 1],
                )

            nc.sync.dma_start(out=do[:], in_=dsum[:])
    nc.compile()
    return nc


_NC = None


def _get_nc():
    global _NC
    if _NC is None:
        _NC = _build_nc()
    return _NC


def _run(inputs, trace=False):
    logits = np.asarray(inputs["logits"], dtype=np.float32).reshape(N, C)
    feats = np.asarray(inputs["feats"], dtype=np.float32).reshape(N, D)
    centers = np.ascontiguousarray(np.asarray(inputs["centers"], dtype=np.float32))
    in_maps = [
        {
            "logits": np.ascontiguousarray(logits[c * ROWS:(c + 1) * ROWS]),
            "feats": np.ascontiguousarray(feats[c * ROWS:(c + 1) * ROWS]),
            "centers": centers,
        }
        for c in range(N_CORES)
    ]
    res = run_bass_kernel_spmd(_get_nc(), in_maps, list(range(N_CORES)), trace=trace)
    # dout[p, t] holds sample t*128+p; transpose -> sample order
    d = np.concatenate([r["dout"].T.reshape(-1) for r in res.results])
    total = np.clip(d.astype(np.float64), CLIP_MIN, CLIP_MAX).sum()
    total += float(N) * (C - 1) * CLIP_MIN
    loss = np.float32(total / N)
    return np.asarray(loss, dtype=np.float32), res


def kernel(**inputs):
    loss, _ = _run(inputs, trace=False)
    return loss



# revision 5
# speedup vs baseline: 1.0238x; 1.0238x over previous
"""CenterLoss kernel for Trainium2, data-parallel over 8 NeuronCores.

loss = sum(clip(distmat * onehot(argmax(logits)), 1e-12, 1e12)) / N
     = (sum_i clip(||f_i - c_{label_i}||^2, 1e-12, 1e12) + N*(C-1)*1e-12) / N

Per core (640 rows): logits rows are host-padded to 6656 cols (-1e38 pad)
= 52 blocks of 128. A column-max "frame" FM[p, t, w] = max_j lg[row, j*128+w]
is built by three parallel routes:
  - gpsimd accum-max DMAs fold blocks directly in the DMA datapath,
  - DVE folds f32 blocks loaded on the sync/scalar HWDGE queues,
  - DVE folds bf16 copies converted on the scalar engine (2x DVE rate;
    bf16 rounding only risks near-tie argmax flips, harmless at rtol 2e-2).
Then per row: o* = argmax of the 128-wide frame, an indirect gather reads
the 52 candidate logits {j*128+o*}, j* = their argmax, label = j**128+o*;
centers row gathered by label (host-padded to 128 cols), distance reduced
via Square-activation accumulate. Host sums the 8x640 distances.
"""

import numpy as np

import concourse.bacc as bacc
import concourse.bass as bass
import concourse.tile as tile
from concourse import mybir
from concourse.bass_utils import run_bass_kernel_spmd

P = 128            # SBUF partitions
C = 6625           # num classes
CP = 6656          # padded row width (52 * 128)
W = 128            # block width
NB = CP // W       # 52 blocks per row
D = 96             # feat dim
CPAD = 128         # padded centers row width
T = 5              # 128-row tiles per core
ROWS = P * T       # 640 samples per core
N_CORES = 8
N = ROWS * N_CORES
CLIP_MIN = 1e-12
CLIP_MAX = 1e12
NEG = -1e38

f32 = mybir.dt.float32
bf16 = mybir.dt.bfloat16
u32 = mybir.dt.uint32
OP = mybir.AluOpType
AF = mybir.ActivationFunctionType

# route split over the 52 blocks (from LP over the CoreSim cost model):
# gpsimd fold-DMA blocks, scalar->bf16-convert blocks, plain f32 DVE blocks
NF = 17
NCV = 16
NDV = NB - NF - NCV  # 19


def _bcast8(ap_col):
    """[P,1] AP -> [P,8] stride-0 broadcast view (for max_index in_max)."""
    return bass.AP(ap_col.tensor, ap_col.offset, [ap_col.ap[0], [0, 8]])


def _flat2(ap3, n):
    """[P,a,b] tile AP -> [P, n] flattened view."""
    return bass.AP(ap3.tensor, ap3.offset, [ap3.ap[0], [1, n]])


def _build_nc():
    nc = bacc.Bacc(None)
    lg = nc.dram_tensor("logits", [ROWS, CP], f32, kind="ExternalInput")
    ft = nc.dram_tensor("feats", [ROWS, D], f32, kind="ExternalInput")
    ct = nc.dram_tensor("centers", [C, CPAD], f32, kind="ExternalInput")
    do = nc.dram_tensor("dout", [P, T], f32, kind="ExternalOutput")

    def lg_blocks(b0, nblk):
        # [128, T, nblk*W] view: rows of all 5 tiles, cols [b0*W, (b0+nblk)*W)
        return bass.AP(lg, b0 * W, [[CP, P], [P * CP, T], [1, nblk * W]])

    # block-index ranges per route
    fold_blocks = list(range(0, NF))
    cv_blocks = list(range(NF, NF + NCV))
    dv_blocks = list(range(NF + NCV, NB))
    # chunking of load routes (blocks per DMA)
    def chunks(blks, sz):
        return [blks[i:i + sz] for i in range(0, len(blks), sz)]
    cv_chunks = chunks(cv_blocks, 4)          # 4 chunks of 4
    dv_chunks = chunks(dv_blocks, 4)          # 4,4,4,4,3
    # sync loads all cv chunks + first 2 dve chunks; scalar loads the rest
    sync_chunks = [("cv", c) for c in cv_chunks] + [("dv", c) for c in dv_chunks[:2]]
    act_chunks = [("dv", c) for c in dv_chunks[2:]]

    with tile.TileContext(nc) as tc:
        with (
            tc.tile_pool(name="big", bufs=4) as big,
            tc.tile_pool(name="persist", bufs=1) as persist,
        ):
            # ---- setup (pool) ----
            gm = persist.tile([P, 8], f32)
            nc.gpsimd.memset(gm[:], NEG)
            gmc = persist.tile([P, 8], f32)
            nc.gpsimd.memset(gmc[:], NEG)
            rowbase = persist.tile([P, T], u32)
            nc.gpsimd.iota(rowbase[:], [[P * CP, T]], base=0, channel_multiplier=CP)
            jio = persist.tile([P, NB], u32)
            nc.gpsimd.iota(jio[:], [[W, NB]], base=0, channel_multiplier=0)
            c128 = persist.tile([P, 1], u32)
            nc.gpsimd.memset(c128[:], W)

            fm = persist.tile([P, T, W], f32)    # f32 frame (DVE)
            fmb = persist.tile([P, T, W], bf16)  # bf16 frame (DVE)
            fmp = persist.tile([P, T, W], f32)   # fold frame (gpsimd DMA)

            F = persist.tile([P, T, D], f32)
            ft3 = bass.AP(ft, 0, [[D, P], [P * D, T], [1, D]])
            nc.sync.dma_start(out=F[:], in_=ft3)

            # ---- fold route: accum-max DMAs on gpsimd ----
            for i, b in enumerate(fold_blocks):
                nc.gpsimd.dma_start(
                    out=fmp[:], in_=lg_blocks(b, 1),
                    accum_op=(OP.bypass if i == 0 else OP.max),
                )

            # ---- load + scan routes ----
            # interleave issue: sync chunk, act chunk, ... convert+DVE per chunk
            order = []
            si, ai = 0, 0
            while si < len(sync_chunks) or ai < len(act_chunks):
                if si < len(sync_chunks):
                    order.append(("sync", sync_chunks[si])); si += 1
                if ai < len(act_chunks):
                    order.append(("act", act_chunks[ai])); ai += 1
            def fold_into(frame, buf, nblk, started):
                """DVE-fold buf's nblk W-wide blocks into frame; returns True
                once frame holds valid data."""
                k = 0
                if not started:
                    if nblk >= 2:
                        nc.vector.tensor_tensor(
                            out=frame[:], in0=buf[:, :, 0:W], in1=buf[:, :, W:2 * W],
                            op=OP.max)
                        k = 2
                    else:
                        nc.vector.tensor_copy(frame[:], buf[:, :, 0:W])
                        k = 1
                while k < nblk:
                    nc.vector.tensor_tensor(
                        out=frame[:], in0=frame[:],
                        in1=buf[:, :, k * W:(k + 1) * W], op=OP.max)
                    k += 1
                return True

            fm_started = False
            fmb_started = False
            for qname, (kind, blks) in order:
                nblk = len(blks)
                buf = big.tile([P, T, nblk * W], f32, tag="ld")
                eng = nc.sync if qname == "sync" else nc.scalar
                eng.dma_start(out=buf[:], in_=lg_blocks(blks[0], nblk))
                if kind == "dv":
                    fm_started = fold_into(fm, buf, nblk, fm_started)
                else:  # cv
                    cvb = big.tile([P, T, nblk * W], bf16, tag="cv")
                    nc.scalar.copy(out=cvb[:], in_=buf[:])
                    fmb_started = fold_into(fmb, cvb, nblk, fmb_started)

            # ---- merge frames + per-row argmax ----
            nc.vector.tensor_tensor(out=fm[:], in0=fm[:], in1=fmp[:], op=OP.max)
            nc.vector.tensor_tensor(out=fm[:], in0=fm[:], in1=fmb[:], op=OP.max)
            nc.vector.reduce_max(gm[:, 0:T], fm[:], axis=mybir.AxisListType.X)

            ost = persist.tile([P, T], u32)   # o* per tile (via per-tile searches)
            base = persist.tile([P, T], u32)
            oi8s = []
            for t in range(T):
                oi8 = persist.tile([P, 8], u32, name=f"oi8_{t}")
                nc.vector.max_index(oi8[:], _bcast8(gm[:, t:t + 1]), fm[:, t, :])
                oi8s.append(oi8)
                nc.gpsimd.tensor_copy(ost[:, t:t + 1], oi8[:, 0:1])
                nc.gpsimd.tensor_add(base[:, t:t + 1], rowbase[:, t:t + 1], oi8[:, 0:1])

            cidx = persist.tile([P, T, NB], u32)
            b3 = bass.AP(base[:].tensor, base[:].offset,
                         [base[:].ap[0], [base[:].ap[-1][0], T], [0, NB]])
            j3 = bass.AP(jio[:].tensor, jio[:].offset,
                         [jio[:].ap[0], [0, T], [1, NB]])
            nc.gpsimd.tensor_add(cidx[:], b3, j3)

            CD = persist.tile([P, T, NB], f32)
            lgflat = bass.AP(lg, 0, [[1, ROWS * CP], [1, 1]])
            nc.gpsimd.indirect_dma_start(
                out=CD[:], out_offset=None, in_=lgflat,
                in_offset=bass.IndirectOffsetOnAxis(ap=_flat2(cidx[:], T * NB), axis=0),
            )
            nc.vector.reduce_max(gmc[:, 0:T], CD[:], axis=mybir.AxisListType.X)

            label = persist.tile([P, T], u32)
            tmpm = persist.tile([P, T], u32)
            for t in range(T):
                ji8 = persist.tile([P, 8], u32, name=f"ji8_{t}")
                nc.vector.max_index(ji8[:], _bcast8(gmc[:, t:t + 1]), CD[:, t, :])
                nc.gpsimd.tensor_mul(tmpm[:, t:t + 1], ji8[:, 0:1], c128[:])
                nc.gpsimd.tensor_add(label[:, t:t + 1], tmpm[:, t:t + 1], ost[:, t:t + 1])

            CR = persist.tile([P, T, CPAD], f32)
            nc.gpsimd.indirect_dma_start(
                out=CR[:], out_offset=None, in_=ct[:],
                in_offset=bass.IndirectOffsetOnAxis(ap=label[:], axis=0),
            )

            DF = persist.tile([P, T, D], f32)
            nc.gpsimd.tensor_sub(DF[:], F[:], CR[:, :, 0:D])
            dsum = persist.tile([P, T], f32)
            SQ = persist.tile([P, T, D], f32)
            for t in range(T):
                nc.scalar.activation(
                    out=SQ[:, t, :], in_=DF[:, t, :], func=AF.Square,
                    accum_out=dsum[:, t:t + 1],
                )

            nc.sync.dma_start(out=do[:], in_=dsum[:])
    nc.compile()
    return nc


_NC = None


def _get_nc():
    global _NC
    if _NC is None:
        _NC = _build_nc()
    return _NC


def _prep(inputs):
    logits = np.asarray(inputs["logits"], dtype=np.float32).reshape(N, C)
    feats = np.asarray(inputs["feats"], dtype=np.float32).reshape(N, D)
    centers = np.asarray(inputs["centers"], dtype=np.float32)
    lg_pad = np.full((N, CP), NEG, dtype=np.float32)
    lg_pad[:, :C] = logits
    ct_pad = np.zeros((C, CPAD), dtype=np.float32)
    ct_pad[:, :D] = centers
    in_maps = [
        {
            "logits": np.ascontiguousarray(lg_pad[c * ROWS:(c + 1) * ROWS]),
            "feats": np.ascontiguousarray(feats[c * ROWS:(c + 1) * ROWS]),
            "centers": ct_pad,
        }
        for c in range(N_CORES)
    ]
    return in_maps


def _run(inputs, trace=False):
    in_maps = _prep(inputs)
    res = run_bass_kernel_spmd(_get_nc(), in_maps, list(range(N_CORES)), trace=trace)
    # dout[p, t] holds sample t*128+p; transpose -> sample order
    d = np.concatenate([r["dout"].T.reshape(-1) for r in res.results])
    total = np.clip(d.astype(np.float64), CLIP_MIN, CLIP_MAX).sum()
    total += float(N) * (C - 1) * CLIP_MIN
    loss = np.float32(total / N)
    return np.asarray(loss, dtype=np.float32), res


def kernel(**inputs):
    loss, _ = _run(inputs, trace=False)
    return loss
